# revision 16
# baseline (speedup 1.0000x reference)
"""Trainium2 Bass kernel for CNODE (dense MLP neural-ODE with masked updates).

Model (see harness reference): B=4096 batch, T=100 obs steps. Per obs step:
4 Euler steps of h += dt*MLP(h) with MLP = relu(relu(h@W1+b1)@W2+b2)@W3+b3,
then y_pred = h[:,0], h_upd = [y_t, h[:,:-1]], h = mask*h_upd + (1-mask)*h.

Strategy: pure data parallel over 8 cores (512 batch each), 2 streams of 256
per core. On-chip state is the transposed h [65, 256]. The pre-activation
P1 = W1^T h lives in PSUM and is updated across Euler steps via the
precomputed W31 = (dt*W3)@W1, so each Euler step costs 3 matmuls + 2
relu-with-bias elementwise ops. h_cn is reconstructed from a second PSUM
accumulator P3 = h0 + sum_k (dt*W3)^T A2_k. The observation update (shift +
y injection + mask blend) is 2 small matmuls + 1 DVE multiply; the blended
state is never materialized in SBUF (it feeds the next step's PSUM init
directly as two matmuls over Hcn and dm = mask*(h_upd - h_cn)).

All matmuls run in float32r (TF32-like, full PE rate at N>=256, ~1.7e-4
relative rounding), accumulation in fp32 PSUM.
"""

import contextlib

import numpy as np

B, T = 4096, 100
NCC = 32
HID = 128
DIM = 2 * NCC + 1  # 65
DELTA_T = 0.25
STEPS = 4
NCORES = 8
BS = B // NCORES  # 512 per core
S = 2  # streams per core
FD = BS // S  # 256


def _build(T_steps, reps=1):
    import concourse.bacc as bacc
    import concourse.mybir as mybir
    import concourse.tile as tile

    F32 = mybir.dt.float32
    F32R = mybir.dt.float32r
    AF = mybir.ActivationFunctionType
    OP = mybir.AluOpType

    nc = bacc.Bacc(target_bir_lowering=False)

    # Inputs (per-core shards for Yt/Mb, replicated consts otherwise)
    yt_d = nc.dram_tensor("Yt", [T_steps, BS], F32R, kind="ExternalInput")
    mb_d = nc.dram_tensor("Mb", [T_steps, DIM, BS], F32, kind="ExternalInput")
    zz_d = nc.dram_tensor("Zz", [DIM, BS], F32R, kind="ExternalInput")
    w1_d = nc.dram_tensor("W1", [DIM, HID], F32R, kind="ExternalInput")
    w2_d = nc.dram_tensor("W2", [HID, HID], F32R, kind="ExternalInput")
    w31_d = nc.dram_tensor("W31", [HID, HID], F32R, kind="ExternalInput")
    w3p_d = nc.dram_tensor("W3p", [HID, DIM], F32R, kind="ExternalInput")
    i65_d = nc.dram_tensor("I65", [DIM, DIM], F32R, kind="ExternalInput")
    ssub_d = nc.dram_tensor("Ssub", [DIM, DIM], F32R, kind="ExternalInput")
    e0_d = nc.dram_tensor("E0", [1, DIM], F32R, kind="ExternalInput")
    b1k_d = nc.dram_tensor("B1K", [HID, STEPS], F32, kind="ExternalInput")
    b2c_d = nc.dram_tensor("B2C", [HID, 1], F32, kind="ExternalInput")
    c4_d = nc.dram_tensor("C4", [DIM, 1], F32, kind="ExternalInput")

    # Outputs
    yp_d = nc.dram_tensor("Ypred", [T_steps, BS], F32, kind="ExternalOutput")
    ho_d = nc.dram_tensor("Hout", [DIM, BS], F32, kind="ExternalOutput")
    hc_d = nc.dram_tensor("Hcnl", [DIM, BS], F32, kind="ExternalOutput")

    with tile.TileContext(nc) as tc:
        with (
            tc.tile_pool(name="const", bufs=1) as cpool,
            tc.tile_pool(name="state", bufs=1) as spool,
            tc.tile_pool(name="mb", bufs=6) as mbpool,
            tc.tile_pool(name="dm", bufs=3) as dmpool,
            tc.tile_pool(name="psum", bufs=1, space="PSUM") as ppool,
        ):
            w1 = cpool.tile([DIM, HID], F32R)
            w2 = cpool.tile([HID, HID], F32R)
            w31 = cpool.tile([HID, HID], F32R)
            w3p = cpool.tile([HID, DIM], F32R)
            i65 = cpool.tile([DIM, DIM], F32R)
            ssub = cpool.tile([DIM, DIM], F32R)
            e0 = cpool.tile([1, DIM], F32R)
            b1k = cpool.tile([HID, STEPS], F32)
            b2c = cpool.tile([HID, 1], F32)
            c4 = cpool.tile([DIM, 1], F32)
            for tl, dr in (
                (w1, w1_d), (w2, w2_d), (w31, w31_d), (w3p, w3p_d),
                (i65, i65_d), (ssub, ssub_d), (e0, e0_d), (b1k, b1k_d),
                (b2c, b2c_d), (c4, c4_d),
            ):
                nc.sync.dma_start(tl[:], dr[:])

            g0 = [
                spool.tile([DIM, FD], F32R, name=f"g0_{s}", tag=f"g0_{s}")
                for s in range(S)
            ]
            a1 = [
                [
                    spool.tile(
                        [HID, FD], F32R, name=f"a1_{s}_{i}", tag=f"a1_{s}_{i}"
                    )
                    for i in range(2)
                ]
                for s in range(S)
            ]
            a2 = [
                [
                    spool.tile(
                        [HID, FD], F32R, name=f"a2_{s}_{i}", tag=f"a2_{s}_{i}"
                    )
                    for i in range(2)
                ]
                for s in range(S)
            ]
            hcn = [
                spool.tile([DIM, FD], F32R, name=f"hcn_{s}", tag=f"hcn_{s}")
                for s in range(S)
            ]
            hfin = [
                spool.tile([DIM, FD], F32R, name=f"hf_{s}", tag=f"hf_{s}")
                for s in range(S)
            ]
            p1 = [
                ppool.tile([HID, FD], F32, name=f"p1_{s}", tag=f"p1_{s}")
                for s in range(S)
            ]
            p2 = [
                ppool.tile([HID, FD], F32, name=f"p2_{s}", tag=f"p2_{s}")
                for s in range(S)
            ]
            p3 = [
                ppool.tile([DIM, FD], F32, name=f"p3_{s}", tag=f"p3_{s}")
                for s in range(S)
            ]
            pd = [
                ppool.tile([DIM, FD], F32, name=f"pd_{s}", tag=f"pd_{s}")
                for s in range(S)
            ]

            for s in range(S):
                sl = slice(s * FD, (s + 1) * FD)
                nc.sync.dma_start(g0[s][:], zz_d[:, sl])

            def relu_bias(s, out, in_, bias_ap):
                if s == 0:
                    nc.scalar.activation(out, in_, AF.Relu, bias=bias_ap)
                else:
                    nc.vector.tensor_scalar(
                        out, in_, bias_ap, 0.0, OP.add, OP.max
                    )

            def emit_body():
                for t in range(T_steps):
                    for s in range(S):
                        sl = slice(s * FD, (s + 1) * FD)
                        mb = mbpool.tile(
                            [DIM, FD], F32, name=f"mb{s}_{t}", tag=f"mb{s}"
                        )
                        nc.sync.dma_start(mb[:], mb_d[t, :, sl])
                        yrow = mbpool.tile(
                            [1, FD], F32R, name=f"y{s}_{t}", tag=f"y{s}"
                        )
                        nc.sync.dma_start(yrow[:], yt_d[t : t + 1, sl])

                        if t == 0:
                            nc.tensor.matmul(
                                p1[s][:], w1[:], g0[s][:],
                                start=True, stop=False,
                            )
                        for k in range(STEPS):
                            a1t = a1[s][k % 2]
                            a2t = a2[s][k % 2]
                            relu_bias(s, a1t[:], p1[s][:], b1k[:, k : k + 1])
                            nc.tensor.matmul(
                                p2[s][:], w2[:], a1t[:], start=True, stop=True
                            )
                            if k == 1:
                                # y injection into Pd (row 0 <- y_t)
                                nc.tensor.matmul(
                                    pd[s][:], e0[:], yrow[:],
                                    start=True, stop=False,
                                )
                            relu_bias(s, a2t[:], p2[s][:], b2c[:, 0:1])
                            if k < STEPS - 1:
                                nc.tensor.matmul(
                                    p1[s][:], w31[:], a2t[:],
                                    start=False, stop=(k == STEPS - 2),
                                )
                            nc.tensor.matmul(
                                p3[s][:], w3p[:], a2t[:],
                                start=(t == 0 and k == 0),
                                stop=(k == STEPS - 1),
                            )
                        # h_cn = P3 + 4*dt*b3
                        if s == 0:
                            nc.scalar.activation(
                                hcn[s][:], p3[s][:], AF.Identity,
                                bias=c4[:, 0:1],
                            )
                        else:
                            nc.vector.tensor_scalar(
                                hcn[s][:], p3[s][:], c4[:, 0:1], None, OP.add
                            )
                        # y_pred out
                        nc.sync.dma_start(
                            yp_d[t : t + 1, sl], hcn[s][0:1, :].bitcast(F32)
                        )
                        # D = shift(h_cn) + e0*y - h_cn  (accumulate into Pd)
                        nc.tensor.matmul(
                            pd[s][:], ssub[:], hcn[s][:], start=False, stop=True
                        )
                        if t < T_steps - 1:
                            # open next P1/P3 groups from h_new = hcn + dm
                            nc.tensor.matmul(
                                p1[s][:], w1[:], hcn[s][:],
                                start=True, stop=False,
                            )
                            nc.tensor.matmul(
                                p3[s][:], i65[:], hcn[s][:],
                                start=True, stop=False,
                            )
                        # dm = mask * D
                        dm = dmpool.tile(
                            [DIM, FD], F32R, name=f"dm{s}_{t}", tag=f"dm{s}"
                        )
                        nc.vector.tensor_tensor(
                            dm[:], mb[:], pd[s][:].bitcast(F32), OP.mult
                        )
                        if t < T_steps - 1:
                            nc.tensor.matmul(
                                p1[s][:], w1[:], dm[:], start=False, stop=False
                            )
                            nc.tensor.matmul(
                                p3[s][:], i65[:], dm[:], start=False, stop=False
                            )
                        else:
                            # final h = hcn + dm, h_cn_last = hcn
                            nc.gpsimd.tensor_tensor(
                                hfin[s][:], hcn[s][:].bitcast(F32),
                                dm[:].bitcast(F32), OP.add,
                            )
                            nc.sync.dma_start(
                                ho_d[:, sl], hfin[s][:].bitcast(F32)
                            )
                            nc.sync.dma_start(
                                hc_d[:, sl], hcn[s][:].bitcast(F32)
                            )

            if reps > 1:
                with tc.For_i(0, reps, 1):
                    emit_body()
            else:
                emit_body()

    nc.compile()
    return nc


def _host_inputs(times, Y, mask, W1, b1, W2, b2, W3, b3, T_steps):
    f32 = np.float32
    W1 = np.asarray(W1, f32)
    b1 = np.asarray(b1, f32)
    W2 = np.asarray(W2, f32)
    b2 = np.asarray(b2, f32)
    W3 = np.asarray(W3, f32)
    b3 = np.asarray(b3, f32)
    Y = np.asarray(Y, f32)
    mask = np.asarray(mask, f32)

    w3p = (DELTA_T * W3).astype(f32)  # [HID, DIM]
    w31 = (w3p @ W1).astype(f32)  # [HID, HID]
    b1k = np.stack(
        [b1 + k * DELTA_T * (b3 @ W1) for k in range(STEPS)], axis=1
    ).astype(f32)  # [HID, STEPS]
    b2c = b2[:, None].astype(f32)
    c4 = (STEPS * DELTA_T * b3)[:, None].astype(f32)
    ssub = (np.diag(np.ones(DIM - 1, f32), 1) - np.eye(DIM, dtype=f32)).astype(
        f32
    )
    e0 = np.zeros((1, DIM), f32)
    e0[0, 0] = 1.0
    i65 = np.eye(DIM, dtype=f32)
    zz = np.zeros((DIM, BS), f32)

    const = {
        "W1": W1, "W2": W2, "W31": w31, "W3p": w3p, "I65": i65,
        "Ssub": ssub, "E0": e0, "B1K": b1k, "B2C": b2c, "C4": c4, "Zz": zz,
    }
    in_maps = []
    for c in range(NCORES):
        bsl = slice(c * BS, (c + 1) * BS)
        yt = np.ascontiguousarray(Y[bsl, :T_steps, 0].T)  # [T, BS]
        msh = mask[bsl, :T_steps].T  # [T, BS]
        mbc = np.ascontiguousarray(
            np.broadcast_to(msh[:, None, :], (T_steps, DIM, BS))
        )
        m = dict(const)
        m["Yt"] = yt
        m["Mb"] = mbc
        in_maps.append(m)
    return in_maps


_NC_CACHE = {}


def _get_nc(T_steps, reps=1):
    key = (T_steps, reps)
    if key not in _NC_CACHE:
        _NC_CACHE[key] = _build(T_steps, reps)
    return _NC_CACHE[key]


def kernel(times, Y, mask, W1, b1, W2, b2, W3, b3, _T_steps=T, _trace=False):
    from concourse.bass_utils import run_bass_kernel_spmd

    T_steps = _T_steps
    nc = _get_nc(T_steps)
    in_maps = _host_inputs(times, Y, mask, W1, b1, W2, b2, W3, b3, T_steps)
    res = run_bass_kernel_spmd(
        nc, in_maps, core_ids=list(range(NCORES)), trace=_trace
    )

    f32 = np.float32
    y_preds = np.empty((B, T_steps, 1), f32)
    h = np.empty((B, DIM), f32)
    hcn = np.empty((B, DIM), f32)
    for c in range(NCORES):
        out = res.results[c]
        bsl = slice(c * BS, (c + 1) * BS)
        y_preds[bsl, :, 0] = out["Ypred"].T
        h[bsl] = out["Hout"].T
        hcn[bsl] = out["Hcnl"].T

    times_traj = np.asarray(times, f32)
    ret = (y_preds, y_preds, times_traj, h, hcn)
    if _trace:
        return ret, res
    return ret
